# revision 95
# baseline (speedup 1.0000x reference)
"""Trainium2 Bass kernel for DiscreteGCNLayer — fp8 DoubleRow version.

Computation (per batch b):
    dw      = ternary_quantize(weight, s=0.01)            # [256, 256]
    support = x[b] @ dw                                   # [2048, 256]
    out[b]  = relu(adj[b] @ support + bias)               # [2048, 256]

Strategy: data-parallel over the batch dim (8 batches -> 8 NeuronCores).
The kernel is memory-bound on adj (the only O(N^2) tensor), so adj is
staged in HBM as ONE byte per element and every stage-2 matmul runs in
fp8 DoubleRow mode (0.5 cycles/row, two 128-deep k-tiles per
instruction -> 4x the bf16 matmul throughput).

fp8 precision plan (hardware-measured rel err 1.65e-2 vs the 2e-2 gate):
  - adj is staged as fp8 of (adj - 0.5) * 32.  The 0.5 shift halves the
    quantization noise (adj is uniform[0,1)); the exactly-representable
    rank-1 term 0.5 * colsum(support) is added back via the per-partition
    bias of the final activation.
  - x is staged as a host-side fp8 (hi, lo*16) pair so stage 1 is
    DoubleRow too; the ternary dw is scaled to +-0.625 (fp8-exact)
    making the stage-1 psum = support * 62.5, and dw for the x-lo
    matmul is +-0.625/16 (also fp8-exact).  x's quantization therefore
    contributes ~fp8^2 error.
  - support: stage-1 psum is converted to fp8 pairs s_hi (ACT) and
    s_lo = fp8(psum - s_hi) (DVE).  Stage 2 uses s_hi everywhere and
    adds the s_lo matmuls for the front-loaded k-pairs in LO_S2_QS.
  - colsum(support) is computed on the PE from the s_hi AND s_lo tiles
    (ones-vector DoubleRow matmuls): the rank-1 0.5*J@support term is a
    per-column CONSTANT, so a colsum of the quantized support alone
    would add a ~2e-2 random-walk column offset; s_lo cancels it.
  - final eviction: out = relu(psum2 * (1/2000) + 0.5/62.5 * colsum + b)
    fused in one ACT activation (or a two-op DVE pair) per output half.

HBM traffic per core: adj 4.19MB + x 1.05MB + w 0.13MB + out 1.05MB
= 6.4MB (vs 10.5MB for the bf16 kernel).  Schedule notes:
  - inputs stream gapless on the SP HWDGE queue in consumption order,
    few large transfers (>=512B contiguous lines); output stores also
    ride SP and naturally defer behind the input stream, landing in the
    kernel tail while the last block's close chain runs.
  - psum->fp8: s_hi on ACT, s_lo on DVE (only DVE can tensor_tensor
    from PSUM; GPSIMD cannot access PSUM at all), chunk-pair batched to
    amortize engine access latency.
  - psum start_tensor_calc pending-zeroes a whole 2KB bank: tiles that
    pack two accumulation groups into one bank (stage-1 chunk pairs,
    the colsum tile, the narrow tail blocks) carry start=True only on
    the very first matmul.
  - the last 512 output columns are two narrow 256-col blocks so the
    post-stream close chain (q7 matmuls -> eviction -> HWDGE gen ->
    store) operates on half-size tiles.  Their bias is folded into the
    psum on the PE via diag(bias) DoubleRow matmuls (diag built as
    (p - o == 0) mask times the per-partition bias column, an fp8 hi/lo
    pair against a constant-column rhs), so each narrow close is ONE
    bias-free ACT activation over both output halves.
"""

import sys

import numpy as np

if "/opt/trn_rl_repo" not in sys.path:
    sys.path.insert(0, "/opt/trn_rl_repo")

B = 8
N = 2048
DIN = 256
DOUT = 256
P = 128
NBW = 512          # stage-2 psum width (n columns per group)
NWIDE = 3          # wide 512-col n-blocks (cols 0:1536)
NBN = 256          # the last 512 cols are TWO narrow blocks: their
NNARROW = 2        # (tail-critical) evictions + stores are half-size
MB = N // P        # 16 m-chunks (stage-2 contraction)
IB = DIN // P      # 2 i-chunks (stage-1 contraction)
NQ2 = MB // 2      # 8 DoubleRow k-pairs per stage-2 group
OH = DOUT // P     # 2 output-partition halves
SPARSITY = 0.01
WSCALE = 0.625     # fp8-exact ternary magnitude; support scale = 62.5
WSCALE_LO = 0.0625     # dw8_lo = dw8 * this (= 0.0390625, fp8-exact)
ASCALE = 32.0      # adj residual scale (power of two)
DESCALE = 1.0 / 2000.0     # 1 / (62.5 * 32)
CSCOMB = 0.5 / 62.5        # colsum -> bias_eff factor
WARMUP = 6
# stage-2 k-pairs (per wide n-block) that also run the support
# lo-residual matmuls: they cut the support quantization error to
# ~sqrt(0.7) (measured ~1.65e-2 total vs the 2e-2 gate).  They are
# front-loaded into nb0/nb1, whose matmuls run while the DMA stream is
# still ahead of the PE; later blocks stay hi-only so the end-of-stream
# PE chain (which sets the kernel tail) is as short as possible.  The
# lo residual itself is produced for every chunk because the colsum
# correction needs it.
LO_S2_QS = {0: frozenset(range(8)), 1: frozenset({0, 4}),
            2: frozenset()}

_NC = None


def _build_nc():
    from contextlib import ExitStack

    import concourse.bass as bass
    import concourse.mybir as mybir
    import concourse.tile as tile
    from concourse import bacc

    F32 = mybir.dt.float32
    F16 = mybir.dt.float16
    BF16 = mybir.dt.bfloat16
    F8 = mybir.dt.float8e4
    Alu = mybir.AluOpType
    DR = mybir.MatmulPerfMode.DoubleRow
    Relu = mybir.ActivationFunctionType.Relu

    nc = bacc.Bacc()
    xt_d = nc.dram_tensor("xt8", [DIN, N, 2], F8, kind="ExternalInput")
    adjt_d = nc.dram_tensor("adjt", [NWIDE, P, MB, NBW], F8, kind="ExternalInput")
    adjn_d = nc.dram_tensor(
        "adjn", [NNARROW, P, MB, NBN], F8, kind="ExternalInput"
    )
    w_d = nc.dram_tensor("weight", [DIN, DOUT], F16, kind="ExternalInput")
    b_d = nc.dram_tensor("bias", [DOUT], F32, kind="ExternalInput")
    out_d = nc.dram_tensor("out", [DOUT, N], BF16, kind="ExternalOutput")

    with tile.TileContext(nc) as tc, ExitStack() as ctx:
        singles = ctx.enter_context(tc.tile_pool(name="singles", bufs=1))
        ot_pool = ctx.enter_context(tc.tile_pool(name="ot", bufs=5))
        psum_s1 = ctx.enter_context(tc.tile_pool(name="ps1", bufs=3, space="PSUM"))
        psum_s2 = ctx.enter_context(tc.tile_pool(name="ps2", bufs=4, space="PSUM"))
        psum_cs = ctx.enter_context(tc.tile_pool(name="pcs", bufs=1, space="PSUM"))

        # ---- weight first on the SP queue (its quantize chain gates
        # stage 1, which gates everything) ---------------------------------
        w_sb = singles.tile([P, IB, DOUT], F16)
        nc.sync.dma_start(out=w_sb, in_=w_d[:].rearrange("(c p) o -> p c o", p=P))

        # hi/lo interleaved innermost so windowed DMAs stay 3-dim/512B-contig
        xt_sb = singles.tile([P, IB, N, 2], F8)
        xt_r = xt_d[:].rearrange("(c p) m h -> p c m h", p=P)

        def start_xt(c0, c1):
            nc.sync.dma_start(out=xt_sb[:, :, c0:c1, :], in_=xt_r[:, :, c0:c1, :])

        aq = [
            singles.tile([P, MB, NBW], F8, name=f"aq{nb}")
            for nb in range(NWIDE)
        ]
        aqn = [
            singles.tile([P, MB, NBN], F8, name=f"aqn{k}")
            for k in range(NNARROW)
        ]

        def start_aq(nb, m0, m1):
            nc.sync.dma_start(out=aq[nb][:, m0:m1, :], in_=adjt_d[nb, :, m0:m1, :])

        def start_aqn(k, m0, m1):
            nc.sync.dma_start(
                out=aqn[k][:, m0:m1, :], in_=adjn_d[k, :, m0:m1, :]
            )

        # ---- DVE memsets: junk (PE warmup operand), scalar constants,
        # the ones column for the colsum matmuls --------------------------
        junk = singles.tile([P, NBW], BF16)
        nc.vector.memset(junk[:, 0:P], 1.0)
        nc.vector.memset(junk[:, P:NBW], 1.0)
        c_s = singles.tile([P, 9], F32)
        nc.vector.memset(c_s[:, 0:1], SPARSITY)
        nc.vector.memset(c_s[:, 1:2], -SPARSITY)
        nc.vector.memset(c_s[:, 2:3], WSCALE)
        nc.vector.memset(c_s[:, 3:4], CSCOMB)
        nc.vector.memset(c_s[:, 4:5], WSCALE_LO)
        nc.vector.memset(c_s[:, 5:6], 0.0)
        nc.vector.memset(c_s[:, 6:7], DESCALE)
        nc.vector.memset(c_s[:, 7:8], 2000.0 / 128.0)   # bias diag scale
        nc.vector.memset(c_s[:, 8:9], 16.0)             # bias lo-residual scale
        ones8 = singles.tile([P, IB, 1], F8)
        nc.vector.memset(ones8, 1.0)

        # warm the ACT relu path off the critical path
        jrelu = singles.tile([P, 8], BF16)
        nc.scalar.activation(jrelu, junk[:, 0:8], Relu, bias=c_s[:, 5:6])

        # ---- PE warm-up: ~3us of continuous activity reaches full clock
        # while the input DMAs are still in flight -------------------------
        jp = psum_s2.tile([P, NBW], F32, tag="ps2")
        nc.tensor.matmul(
            jp[:, 0:P], lhsT=junk[:, 0:P], rhs=junk[:, 0:P], start=True, stop=True
        )
        for _ in range(WARMUP):
            jp = psum_s2.tile([P, NBW], F32, tag="ps2")
            nc.tensor.matmul(jp, lhsT=junk[:, 0:P], rhs=junk, start=True, stop=True)
        jp = psum_s2.tile([P, NBW], F32, tag="ps2")
        nc.tensor.matmul(
            jp[:, 0:P], lhsT=junk[:, 0:P], rhs=junk[:, 0:P], start=True, stop=True
        )

        # ---- rest of the input stream, SP-queue order = consumption order.
        # Transfers >= the SP SEQ's per-DMA processing time keep the DMA
        # engines gapless; nb3 lands in three pieces so its matmuls overlap
        # the stream and only one k-pair's work trails the last transfer.
        start_xt(0, 1024)         # s1 chunks 0-7
        start_aq(0, 0, 8)         # s2 nb0 q0-3
        start_xt(1024, 2048)      # s1 chunks 8-15
        start_aq(0, 8, 16)        # s2 nb0 q4-7
        bias_sb = singles.tile([P, OH], F32)
        nc.sync.dma_start(out=bias_sb, in_=b_d[:].rearrange("(c p) -> p c", p=P))
        start_aq(1, 0, 8)
        start_aq(1, 8, 16)
        start_aq(2, 0, 8)
        start_aq(2, 8, 12)
        start_aq(2, 12, 16)
        start_aqn(0, 0, 8)
        start_aqn(0, 8, 12)
        start_aqn(0, 12, 16)
        start_aqn(1, 0, 4)
        start_aqn(1, 4, 8)
        start_aqn(1, 8, 12)
        start_aqn(1, 12, 14)
        start_aqn(1, 14, 16)      # small last pieces -> short tail

        # ---- ternary quantize on DVE: dw8 = ((w>s) - (w<-s)) * 0.625 ----
        tpos = singles.tile([P, IB, DOUT], F8)
        tneg = singles.tile([P, IB, DOUT], F8)
        dw8 = singles.tile([P, IB, DOUT], F8)
        dw8lo = singles.tile([P, IB, DOUT], F8)
        nc.vector.tensor_scalar(
            out=tpos, in0=w_sb, scalar1=c_s[:, 0:1], scalar2=c_s[:, 2:3],
            op0=Alu.is_gt, op1=Alu.mult,
        )
        nc.vector.tensor_scalar(
            out=tneg, in0=w_sb, scalar1=c_s[:, 1:2], scalar2=c_s[:, 2:3],
            op0=Alu.is_lt, op1=Alu.mult,
        )
        nc.vector.tensor_sub(dw8, tpos, tneg)
        nc.vector.tensor_scalar(
            out=dw8lo, in0=dw8, scalar1=c_s[:, 4:5], scalar2=None, op0=Alu.mult,
        )

        # ---- stage 1: psum = support * 62.5, converted to fp8 hi + lo ----
        # Processed in chunk PAIRS: one [P, 2, 256] psum tile is exactly one
        # bank, and psum start_tensor_calc pending-zeroes the whole bank, so
        # only the tile's first matmul carries start=True; the second
        # chunk's first write hits still-pending bytes and replaces.
        # Pair-batched fp8 conversions amortize the engines' access latency.
        s_hi = singles.tile([P, MB, DOUT], F8)
        s_lo = singles.tile([P, MB, DOUT], F8)

        def s1_pair(q):
            sp = psum_s1.tile([P, 2, DOUT], F32, tag="ps1")
            for k in range(2):
                mc = 2 * q + k
                nc.tensor.matmul(
                    sp[:, k, :], lhsT=xt_sb[:, :, mc * P : (mc + 1) * P, 0],
                    rhs=dw8, start=(k == 0), stop=False, perf_mode=DR,
                    skip_group_check=True,
                )
                nc.tensor.matmul(
                    sp[:, k, :], lhsT=xt_sb[:, :, mc * P : (mc + 1) * P, 1],
                    rhs=dw8lo, start=False, stop=(k == 1), perf_mode=DR,
                    skip_group_check=True,
                )
            # hi conversions on ACT, lo subs on DVE (only DVE can: GPSIMD
            # cannot touch PSUM, ACT cannot tensor_tensor) — keeping the
            # two chains on separate engines lets the lo chain (which only
            # the colsum correction waits on) trail the hi chain (which
            # stage 2 consumes) without slowing it
            m0 = 2 * q
            nc.scalar.copy(s_hi[:, m0 : m0 + 2, :], sp)
            nc.vector.tensor_sub(
                s_lo[:, m0 : m0 + 2, :], sp, s_hi[:, m0 : m0 + 2, :]
            )

        # ---- colsum via ones-matmuls on the PE (exact w.r.t. stage 2).
        # One [P, 2] tile holds both output halves' accumulators; only the
        # very first matmul carries start=True (it pending-zeroes the whole
        # bank, so oh1's first write lands on pending bytes and replaces —
        # a second start=True would wipe oh0's partial sum).
        cs_t = psum_cs.tile([P, OH], F32, tag="pcs")

        def colsum_pair(q):
            last = q == NQ2 - 1
            for oh in range(OH):
                nc.tensor.matmul(
                    cs_t[:, oh : oh + 1],
                    lhsT=s_hi[:, 2 * q : 2 * q + 2, oh * P : (oh + 1) * P],
                    rhs=ones8, start=(q == 0 and oh == 0), stop=False,
                    perf_mode=DR, skip_group_check=True,
                )
                nc.tensor.matmul(
                    cs_t[:, oh : oh + 1],
                    lhsT=s_lo[:, 2 * q : 2 * q + 2, oh * P : (oh + 1) * P],
                    rhs=ones8, start=False, stop=last, perf_mode=DR,
                    skip_group_check=True,
                )

        # ---- stage 2: psum2[oh] += s8 pairs @ adj pairs ------------------
        s2_psums = {}

        def s2_pair(nb, q, close_order=False):
            if q == 0:
                s2_psums[nb] = [
                    psum_s2.tile([P, NBW], F32, tag="ps2", name=f"po{nb}_{oh}")
                    for oh in range(OH)
                ]
            po = s2_psums[nb]
            pair = aq[nb][:, 2 * q : 2 * q + 2, :]
            last = q == NQ2 - 1
            srcs = [s_hi, s_lo] if q in LO_S2_QS[nb] else [s_hi]
            if not close_order:
                for i, src in enumerate(srcs):
                    for oh in range(OH):
                        nc.tensor.matmul(
                            po[oh],
                            lhsT=src[:, 2 * q : 2 * q + 2, oh * P : (oh + 1) * P],
                            rhs=pair, start=(q == 0 and i == 0),
                            stop=(last and i == len(srcs) - 1), perf_mode=DR,
                        )
            else:
                # close oh0's group first: its (slower, two-op) DVE eviction
                # starts while oh1's last matmul still runs on the PE, so
                # both halves finish together and share one merged store
                for oh in (0, 1):
                    for i, src in enumerate(srcs):
                        nc.tensor.matmul(
                            po[oh],
                            lhsT=src[:, 2 * q : 2 * q + 2, oh * P : (oh + 1) * P],
                            rhs=pair, start=(q == 0 and i == 0),
                            stop=(last and i == len(srcs) - 1), perf_mode=DR,
                        )

        out_r = out_d[:].rearrange("(c p) m -> p c m", p=P)

        def s2_close(nb):
            # oh0 relu+bias+descale fused on ACT; oh1 as two DVE ops in
            # parallel so the close's latency is one engine pass, not two
            po = s2_psums[nb]
            ot = ot_pool.tile([P, OH, NBW], BF16, tag="ot")
            nc.scalar.activation(
                ot[:, 0, :], po[0], Relu,
                bias=bias_eff[:, 0:1], scale=DESCALE,
            )
            t3 = ot_pool.tile([P, NBW], BF16, tag="t3")
            nc.vector.tensor_scalar(
                out=t3, in0=po[1], scalar1=c_s[:, 6:7],
                scalar2=bias_eff[:, 1:2], op0=Alu.mult, op1=Alu.add,
            )
            nc.vector.tensor_scalar(
                out=ot[:, 1, :], in0=t3, scalar1=c_s[:, 5:6], scalar2=None,
                op0=Alu.max,
            )
            nc.sync.dma_start(
                out=out_r[:, :, nb * NBW : (nb + 1) * NBW], in_=ot
            )

        # ---- phase A: stage 1 interleaved with nb0, paced to arrivals ---
        for q in range(4):
            s1_pair(q)
        for q in range(4):
            colsum_pair(q)
        s2_pair(0, 0)
        s2_pair(0, 1)
        s2_pair(0, 2)
        s2_pair(0, 3)
        for q in range(4, 8):
            s1_pair(q)
        for q in range(4, 8):
            colsum_pair(q)
        # bias_eff[o] = 0.5/62.5 * colsum + bias  (per-partition scalars)
        bias_eff = singles.tile([P, OH], F32)
        nc.vector.tensor_scalar(
            out=bias_eff, in0=cs_t,
            scalar1=c_s[:, 3:4], scalar2=None, op0=Alu.mult,
        )
        nc.vector.tensor_add(bias_eff, bias_eff, bias_sb)

        # ---- diag(bias) for the narrow tail blocks: dbias[oh][p, i, o] is
        # an fp8 hi/lo pair carrying bias_eff[o]*2000 on the diagonal, so
        # one DoubleRow matmul against a constant-column rhs folds the bias
        # into the psum and the tail close becomes a single bias-free ACT
        # op over both output halves.  diag construction without partition
        # broadcast: mask[p, o] = (p - o == 0), scaled by the PER-PARTITION
        # scalar bias column (on the diagonal o == p, so row-p's scalar IS
        # bias[o]).
        iot = singles.tile([P, P], mybir.dt.int16)
        nc.gpsimd.iota(iot, pattern=[[-1, P]], channel_multiplier=1)
        mask8 = singles.tile([P, P], F8)
        nc.vector.tensor_scalar(
            out=mask8, in0=iot, scalar1=c_s[:, 5:6], scalar2=None,
            op0=Alu.is_equal,
        )
        onesK = singles.tile([P, IB, NBN], F8)
        nc.vector.memset(onesK[:, 0, :], 128.0)
        nc.vector.memset(onesK[:, 1, :], 8.0)
        dbias = []
        for oh in range(OH):
            bsc = singles.tile([P, 1], F32, name=f"bsc{oh}")
            nc.vector.tensor_scalar(
                out=bsc, in0=bias_eff[:, oh : oh + 1], scalar1=c_s[:, 7:8],
                scalar2=None, op0=Alu.mult,
            )
            rh8 = singles.tile([P, 1], F8, name=f"rh8{oh}")
            nc.vector.tensor_copy(rh8, bsc)
            r16 = singles.tile([P, 1], F32, name=f"r16{oh}")
            nc.vector.tensor_sub(r16, bsc, rh8)
            nc.vector.tensor_scalar(
                out=r16, in0=r16, scalar1=c_s[:, 8:9], scalar2=None,
                op0=Alu.mult,
            )
            db = singles.tile([P, IB, P], F8, name=f"dbias{oh}")
            nc.vector.tensor_scalar(
                out=db[:, 0, :], in0=mask8, scalar1=bsc, scalar2=None,
                op0=Alu.mult,
            )
            nc.vector.tensor_scalar(
                out=db[:, 1, :], in0=mask8, scalar1=r16, scalar2=None,
                op0=Alu.mult,
            )
            dbias.append(db)
        s2_pair(0, 4)
        s2_pair(0, 5)
        s2_pair(0, 6)
        s2_pair(0, 7)
        s2_close(0)

        # ---- steady state: nb1, nb2 --------------------------------------
        for nb in (1, 2):
            for q in range(NQ2):
                s2_pair(nb, q)
            s2_close(nb)

        # ---- narrow tail blocks (cols 1536:2048 as two 256-col blocks):
        # each block's two output-half accumulators live in ONE ps1-pool
        # bank ([P, 2, 256]; start=True only on the very first matmul,
        # the other half's first write lands on pending-zeroed bytes);
        # evictions are half-size (DVE two-op on oh0 starting one matmul
        # early, ACT fused on oh1) and each block stores as one small
        # [128, 2, 256] transfer, so the post-stream chain is short ------
        for k in range(NNARROW):
            pn = psum_s1.tile([P, OH, NBN], F32, tag="ps1", name=f"pn{k}")
            for q in range(NQ2):
                last = q == NQ2 - 1
                for oh in range(OH):
                    nc.tensor.matmul(
                        pn[:, oh, :],
                        lhsT=s_hi[:, 2 * q : 2 * q + 2, oh * P : (oh + 1) * P],
                        rhs=aqn[k][:, 2 * q : 2 * q + 2, :],
                        start=(q == 0 and oh == 0), stop=last, perf_mode=DR,
                        skip_group_check=True,
                    )
                if q == 0:
                    # fold the bias in while the block accumulates
                    for oh in range(OH):
                        nc.tensor.matmul(
                            pn[:, oh, :], lhsT=dbias[oh], rhs=onesK,
                            start=False, stop=False, perf_mode=DR,
                            skip_group_check=True,
                        )
            otn = ot_pool.tile([P, OH, NBN], BF16, tag="otn")
            nc.scalar.activation(
                otn, pn, Relu, bias=c_s[:, 5:6], scale=DESCALE
            )
            off = NWIDE * NBW + k * NBN
            nc.sync.dma_start(out=out_r[:, :, off : off + NBN], in_=otn)

    nc.compile()
    nc._dbg_tensors = {
        "xt8": xt_d, "adjt": adjt_d, "adjn": adjn_d, "weight": w_d,
        "bias": b_d, "out": out_d, "dw8": dw8, "dw8lo": dw8lo,
        "s_hi": s_hi, "bias_eff": bias_eff, "xt_sb": xt_sb,
    }
    return nc


def _get_nc():
    global _NC
    if _NC is None:
        _NC = _build_nc()
    return _NC


def kernel(x, adj, weight, bias, _trace=False):
    import ml_dtypes
    from concourse import bass_utils

    f8 = ml_dtypes.float8_e4m3
    x = np.asarray(x, dtype=np.float32)
    adj = np.asarray(adj, dtype=np.float32)
    weight16 = np.ascontiguousarray(
        np.asarray(weight, dtype=np.float32)
    ).astype(np.float16)
    bias32 = np.ascontiguousarray(np.asarray(bias, dtype=np.float32))

    nc = _get_nc()
    in_maps = []
    for b in range(B):
        xT = np.ascontiguousarray(x[b].T)                    # [256, 2048]
        hi = xT.astype(f8)
        lo = ((xT - hi.astype(np.float32)) * np.float32(16.0)).astype(f8)
        xt8 = np.ascontiguousarray(np.stack([hi, lo], axis=-1))
        A8 = ((adj[b] - np.float32(0.5)) * np.float32(ASCALE)).astype(f8)
        # adjt[nb, p, mc, j] = A8[nb*512 + j, mc*128 + p]  (wide blocks)
        wide = NWIDE * NBW
        adjt = np.ascontiguousarray(
            A8[:wide].reshape(NWIDE, NBW, MB, P).transpose(0, 3, 2, 1)
        )
        # adjn[k, p, mc, j] = A8[wide + k*256 + j, mc*128 + p]
        adjn = np.ascontiguousarray(
            A8[wide:].reshape(NNARROW, NBN, MB, P).transpose(0, 3, 2, 1)
        )
        in_maps.append(
            {"xt8": xt8, "adjt": adjt, "adjn": adjn,
             "weight": weight16, "bias": bias32}
        )

    def _run():
        res = bass_utils.run_bass_kernel_spmd(
            nc, in_maps, core_ids=list(range(B)), trace=_trace
        )
        # materialize inside the guard: results are lazy jax arrays, so a
        # device error only surfaces at np.asarray
        out = np.stack(
            [np.asarray(r["out"]).astype(np.float32).T for r in res.results],
            axis=0,
        )
        return res, out

    try:
        res, out = _run()
    except Exception:
        # one retry: a previously wedged NeuronCore surfaces as a transient
        # NRT_EXEC_UNIT_UNRECOVERABLE on the first execution after it
        res, out = _run()
    if _trace:
        return out, res
    return out
